# revision 68
# baseline (speedup 1.0000x reference)
"""Multi-head self-attention (RoPE, eval-mode) Trainium2 Bass kernel.

Problem: B=2, T=2048, D=1024, H=16, d_head=64, fp32 I/O.

Sharding (8 cores): core c handles batch b=c//4 and the 4 heads
[4g, 4g+4) where g=c%4.  QKV/attention are head-local; the output
projection produces a per-core partial (contraction over this core's
256 head-dims) which the host sums across the 4 cores of each batch
and adds b_out.

Design (v2):
  - All inputs bf16 (PSUM accumulation fp32, output partials bf16,
    host-summed in fp32).  DMAs are full 8KB-per-partition lines.
  - One software-pipelined loop over 8 attention blocks (2 head-pairs
    x 4 tq-blocks of 512 queries) x 16 tk tiles: score matmuls feed
    exp on the scalar engine (the bottleneck, ~135us of ACTIVATE), PV
    consumes exp output one tile behind.  QKV chains, RoPE, V staging
    and outproj are filler inside attention slots.
  - Scores run as K=64 row-group matmul pairs (two heads in disjoint
    32-row strips execute concurrently on the PE).
  - Prefix: dummy exp preloads the ACT table set; junk matmuls keep
    HAM at full clock through the DMA window; tail repeats the trick
    so the last outproj doesn't run at half clock.
"""

import numpy as np

B, T, D = 2, 2048, 1024
H = 16
DH = 64
NCORES = 8
P = 128

ROW_TILED = True  # K=64 row-group score matmuls (2 heads concurrent)

_CACHE = {}


def _rope_tables_np():
    theta = 1.0 / (10000.0 ** (np.arange(0, DH, 2, dtype=np.float32) / DH))
    angles = np.outer(np.arange(T, dtype=np.float32), theta)  # (T, 32)
    angles = np.concatenate([angles, angles], axis=-1)  # (T, DH)
    cos = np.cos(angles).astype(np.float32)
    sin = np.sin(angles).astype(np.float32)
    cosT = np.ascontiguousarray(cos.T)  # (64, T)
    sinT = np.ascontiguousarray(sin.T)
    sinT_signed = np.concatenate([-sinT[0:32], sinT[32:64]], axis=0)
    cos2 = np.tile(cosT, (2, 1))  # (128, T)
    sin2 = np.tile(sinT_signed, (2, 1))
    return cos2, sin2


def _build_module():
    import concourse.mybir as mybir
    import concourse.tile as tile
    from concourse import bacc

    f32 = mybir.dt.float32
    f32r = mybir.dt.float32r
    i32 = mybir.dt.int32
    bf16 = mybir.dt.bfloat16
    # Schraudolph exp(0.125*s) ~= bitcast_f32(round(A*s + B)); ~1.6% RMS.
    # Used for a minority of score tiles on the otherwise-idle DVE so the
    # scalar engine's exp stream (the kernel's roofline) shortens.
    SCHR_A = (2.0 ** 23) / 0.6931471805599453 * 0.125
    SCHR_B = float(1065353216 - 330000)
    # Offloading exp tiles to the DVE measured SLOWER end-to-end (the
    # in-order DVE queue delays the PSUM score reads and stalls the sc
    # rotation), so the set is empty; the machinery is kept for reference.
    SCHR = set()

    nc = bacc.Bacc("TRN2", target_bir_lowering=False, debug=False)
    xT = nc.dram_tensor("xT", [4, P, 8, 512], bf16, kind="ExternalInput")
    w_qk = nc.dram_tensor("w_qk", [4, 2, P, 4, P], bf16, kind="ExternalInput")
    trig = nc.dram_tensor("trig", [P, 4, 2, 512], bf16, kind="ExternalInput")
    wvo = nc.dram_tensor("wvo", [P, 2, 2048], bf16, kind="ExternalInput")
    out = nc.dram_tensor("out", [T, D], bf16, kind="ExternalOutput")

    Exp = mybir.ActivationFunctionType.Exp

    with tile.TileContext(nc) as tc:
        with (
            tc.tile_pool(name="persist", bufs=1) as persist,
            tc.tile_pool(name="expp", bufs=4) as epool,
            tc.tile_pool(name="rope", bufs=2) as rpool,
            tc.tile_pool(name="ob", bufs=3) as opool,
            tc.tile_pool(name="norm", bufs=2) as npool,
            tc.tile_pool(name="sc_ps", bufs=2, space="PSUM") as scps,
            tc.tile_pool(name="pv_ps", bufs=1, space="PSUM") as pvps,
            tc.tile_pool(name="scratch_ps", bufs=2, space="PSUM") as sps,
        ):
            # ---- persistent SBUF ----------------------------------------
            wqk_sb = [
                [
                    persist.tile([P, 4, P], bf16, tag=f"wqk{cc}{i}",
                                 name=f"wqk{cc}{i}")
                    for i in range(2)
                ]
                for cc in range(4)
            ]
            trig_sb = persist.tile([P, 4, 2, 512], bf16)
            wvo_sb = persist.tile([P, 2, 2048], bf16)
            wv_sb = wvo_sb[:, 0, :].rearrange("p (a b) -> p a b", b=256)
            wo_sb = wvo_sb[:, 1, :].rearrange("p (a b) -> p a b", b=1024)
            x_sb = [
                persist.tile([P, 8, 512], bf16, tag=f"x{q}", name=f"x{q}")
                for q in range(4)
            ]
            # roped q / k, two heads stacked on partitions
            q_q = [
                [persist.tile([P, 512], bf16, tag=f"q{hp}_{t}", name=f"q{hp}_{t}")
                 for t in range(4)]
                for hp in range(2)
            ]
            kst = [
                [persist.tile([P, 512], bf16, tag=f"k{hp}_{t}", name=f"k{hp}_{t}")
                 for t in range(4)]
                for hp in range(2)
            ]
            if not ROW_TILED:
                kpad = [
                    [
                        [persist.tile([P, 512], bf16, tag=f"kp{hp}{h}_{t}",
                                      name=f"kp{hp}{h}_{t}") for t in range(4)]
                        for h in range(2)
                    ]
                    for hp in range(2)
                ]
            # per (tk-tile, head): [ones | v] stationary 128x128
            vaug = persist.tile([P, 16, 4, P], bf16)
            attn_q = [
                [persist.tile([P, 512], bf16, tag=f"at{hp}_{b}", name=f"at{hp}_{b}")
                 for b in range(4)]
                for hp in range(2)
            ]
            warm = persist.tile([P, 512], bf16)
            dummy = persist.tile([P, 16], bf16)
            dummy_o = persist.tile([P, 16], bf16)

            # ---- prefix: PE warmup + ACT table preload ------------------
            nc.vector.memset(warm[:], 0.0)

            def warm_mms(n, nn=512):
                for _ in range(n):
                    wps = sps.tile([P, 512], f32, tag="ps", name="ps")
                    nc.tensor.matmul(
                        wps[:, 0:nn], lhsT=warm[:, 0:P], rhs=warm[:, 0:nn],
                        start=True, stop=True,
                    )

            warm_mms(22)
            nc.vector.memset(dummy[:], 0.0)
            nc.scalar.activation(dummy_o[:], dummy[:], Exp, scale=0.125)

            # input DMAs, chunked in need order.  First-needed bytes split
            # across the two fast HWDGE rings (sync + scalar); gpsimd's
            # SWDGE ring starts ~4us late so it only gets late-deadline
            # bulk loads.
            nc.sync.dma_start(wqk_sb[1][0][:], w_qk[1, 0])
            nc.scalar.dma_start(trig_sb[:, 0], trig[:, 0])
            nc.scalar.dma_start(wqk_sb[1][1][:], w_qk[1, 1])
            nc.sync.dma_start(x_sb[0][:, 0:4, :], xT[0, :, 0:4, :])
            nc.scalar.dma_start(x_sb[0][:, 4:8, :], xT[0, :, 4:8, :])
            nc.sync.dma_start(wqk_sb[0][0][:], w_qk[0, 0])
            nc.scalar.dma_start(wqk_sb[0][1][:], w_qk[0, 1])
            nc.sync.dma_start(wvo_sb[:, 0, :], wvo[:, 0, :])
            nc.gpsimd.memset(vaug[:, :, :, 0:64], 1.0)
            nc.scalar.dma_start(trig_sb[:, 1:4], trig[:, 1:4])
            nc.scalar.dma_start(wvo_sb[:, 1, :], wvo[:, 1, :])
            if not ROW_TILED:
                for hp in range(2):
                    for t in range(4):
                        nc.gpsimd.memset(kpad[hp][0][t][64:128, :], 0.0)
                        nc.gpsimd.memset(kpad[hp][1][t][0:64, :], 0.0)

            # ---- building blocks ----------------------------------------
            def fm_chain(cc, t):
                """Feature-major q (cc 0,2) or stacked k (cc 1,3) chain for
                T-quarter t."""
                hp = cc // 2
                dst = (q_q if cc % 2 == 0 else kst)[hp][t]
                ps = sps.tile([P, 512], f32, tag="ps", name="ps")
                for dc in range(8):
                    nc.tensor.matmul(
                        ps[:],
                        lhsT=wqk_sb[cc][dc // 4][:, dc % 4, :],
                        rhs=x_sb[t][:, dc, :],
                        start=(dc == 0),
                        stop=(dc == 7),
                    )
                nc.vector.tensor_copy(dst[:], ps[:])

            def v_chain(tk):
                t, t4 = tk // 4, tk % 4
                ps = sps.tile([P, 512], f32, tag="ps", name="ps")
                psv = ps[:, 0:256]
                for dc in range(8):
                    nc.tensor.matmul(
                        psv,
                        lhsT=x_sb[t][:, dc, t4 * P:(t4 + 1) * P],
                        rhs=wv_sb[:, dc, :],
                        start=(dc == 0),
                        stop=(dc == 7),
                    )
                # all v chains run while ACT still has idle (prefix/block 0),
                # so the PSUM drain rides on the scalar engine and keeps the
                # DVE free for RoPE
                nc.scalar.copy(
                    vaug[:, tk, :, 64:128],
                    psv.rearrange("p (h e) -> p h e", e=64),
                )

            def rope(cc, t):
                """RoPE one T-quarter of q (cc 0,2) or stacked k (cc 1,3),
                in place.  rotate_half = four 32-partition-shifted DVE muls
                (each operand carries its own partition base); the sign is
                folded into the sin table."""
                hp = cc // 2
                base = (q_q if cc % 2 == 0 else kst)[hp][t]
                rot = rpool.tile([P, 512], bf16, tag="rot", name="rot")
                t1 = rpool.tile([P, 512], bf16, tag="t1", name="t1")
                nc.vector.tensor_mul(t1[:], base[:], trig_sb[:, t, 0, :])
                for blk in range(4):
                    s = (blk ^ 1) * 32
                    d = slice(blk * 32, (blk + 1) * 32)
                    nc.vector.tensor_copy(rot[d, :], base[s:s + 32, :])
                nc.vector.tensor_mul(rot[:], rot[:], trig_sb[:, t, 1, :])
                nc.vector.tensor_add(base[:], t1[:], rot[:])
                if not ROW_TILED and cc % 2 == 1:
                    nc.vector.tensor_copy(kpad[hp][0][t][0:64, :], base[0:64, :])
                    nc.vector.tensor_copy(kpad[hp][1][t][64:128, :], base[64:128, :])

            def outproj_unit(b, tqc, tail=False):
                row = b * 4 + tqc
                for d2 in range(2):
                    po = sps.tile([P, 512], f32, tag="ps", name="ps")
                    for hp in range(2):
                        nc.tensor.matmul(
                            po[:],
                            lhsT=attn_q[hp][b][:, tqc * P:(tqc + 1) * P],
                            rhs=wo_sb[:, hp, d2 * 512:(d2 + 1) * 512],
                            start=(hp == 0),
                            stop=(hp == 1),
                        )
                    ob = opool.tile([P, 512], bf16, tag="ob", name="ob")
                    if tail and d2 == 1:
                        # ACT is idle after the last exp; split the PSUM
                        # drains across both engines
                        nc.scalar.copy(ob[:], po[:])
                    else:
                        nc.vector.tensor_copy(ob[:], po[:])
                    nc.sync.dma_start(
                        out[row * P:(row + 1) * P, d2 * 512:(d2 + 1) * 512], ob[:]
                    )

            # remaining bulk DMAs (nothing else contends on these queues now)
            nc.sync.dma_start(x_sb[1][:, 0:4, :], xT[1, :, 0:4, :])
            nc.sync.dma_start(x_sb[1][:, 4:8, :], xT[1, :, 4:8, :])
            nc.gpsimd.dma_start(x_sb[2][:, 0:4, :], xT[2, :, 0:4, :])
            nc.gpsimd.dma_start(x_sb[2][:, 4:8, :], xT[2, :, 4:8, :])
            nc.sync.dma_start(x_sb[3][:, 0:4, :], xT[3, :, 0:4, :])
            nc.sync.dma_start(x_sb[3][:, 4:8, :], xT[3, :, 4:8, :])
            nc.gpsimd.dma_start(wqk_sb[3][0][:], w_qk[3, 0])
            nc.gpsimd.dma_start(wqk_sb[3][1][:], w_qk[3, 1])
            nc.gpsimd.dma_start(wqk_sb[2][0][:], w_qk[2, 0])
            nc.gpsimd.dma_start(wqk_sb[2][1][:], w_qk[2, 1])

            # ---- prefix chains: k00/q00 interleaved dc-by-dc ------------
            psk = sps.tile([P, 512], f32, tag="ps", name="ps")
            psq = sps.tile([P, 512], f32, tag="ps", name="ps")
            for dc in range(8):
                for cc, ps in ((1, psk), (0, psq)):
                    nc.tensor.matmul(
                        ps[:],
                        lhsT=wqk_sb[cc][dc // 4][:, dc % 4, :],
                        rhs=x_sb[0][:, dc, :],
                        start=(dc == 0),
                        stop=(dc == 7),
                    )
            nc.scalar.copy(kst[0][0][:], psk[:])
            nc.scalar.copy(q_q[0][0][:], psq[:])
            rope(1, 0)
            rope(0, 0)
            # v chains for the first quarter fill the PE idle window while
            # the ropes run on the DVE
            for tk in range(4):
                v_chain(tk)

            # ---- slot-planned attention loop ----------------------------
            fillers = {}

            def add(bi, i, fn):
                fillers.setdefault((bi, i), []).append(fn)

            def add_chain(bi, i, cc, t):
                add(bi, i, (lambda: fm_chain(cc, t)))
                add(bi, i + 1, (lambda: rope(cc, t)))

            for i in range(4, 16):
                add(0, i, (lambda tk: lambda: v_chain(tk))(i))
            # chains spread by deadline: k0q roped before iter 4q of block 0,
            # q01->bi1, hp1 k by bi4, q02->bi2, q03->bi3, q1x -> bi4..7
            add_chain(0, 1, 1, 1)
            add_chain(0, 3, 1, 2)
            add_chain(0, 7, 1, 3)
            add_chain(0, 12, 0, 1)
            add_chain(1, 1, 3, 0)
            add_chain(1, 5, 3, 1)
            add_chain(1, 9, 0, 2)
            add_chain(2, 1, 3, 2)
            add_chain(2, 5, 3, 3)
            add_chain(2, 9, 0, 3)
            add_chain(3, 1, 2, 0)
            add_chain(3, 5, 2, 1)
            add_chain(4, 1, 2, 2)
            add_chain(4, 5, 2, 3)
            # outproj units sit early in each 4-iter group so none of them
            # delays the next block's first scores at the boundary
            for b in range(3):
                for tqc in range(4):
                    add(5 + b, 4 * tqc + 1,
                        (lambda bb, tt: lambda: outproj_unit(bb, tt))(b, tqc))

            # PV runs one iteration behind scores/exp so the next block's
            # first scores reach ACT without waiting for this block's last
            # PV pair (keeps the exp stream gap-free at block boundaries).
            pending = None  # (pv_tiles, ex_tile, hp, i, last)

            def emit_pv(p):
                pvt, ex, php, pi, last, schr = p
                for h in range(2):
                    sl = slice(h * 512, (h + 1) * 512)
                    rhs = ex[:, sl]
                    nc.tensor.matmul(
                        pvt[h][:],
                        lhsT=vaug[:, pi, php * 2 + h, :],
                        rhs=rhs,
                        start=(pi == 0),
                        stop=(pi == 15),
                    )

            def emit_norm(php, ptq, pvt):
                for h in range(2):
                    rc = npool.tile([64, 512], f32, tag="rc", name="rc")
                    nc.vector.reciprocal_approx_fast(rc[:], pvt[h][0:64, :])
                    hb = h * 64
                    nc.vector.tensor_mul(
                        attn_q[php][ptq][hb:hb + 64, :], pvt[h][64:128, :], rc[:]
                    )

            for bi in range(8):
                hp, tq = bi // 4, bi % 4
                pv = None
                for i in range(16):
                    sc = scps.tile([P, 1024], f32, tag="sc", name="sc")
                    ko = (i % 4) * P
                    for h in range(2):
                        if ROW_TILED:
                            hsl = slice(h * 64, (h + 1) * 64)
                            nc.tensor.matmul(
                                sc[:, h * 512:(h + 1) * 512],
                                lhsT=kst[hp][i // 4][hsl, ko:ko + P],
                                rhs=q_q[hp][tq][hsl, :],
                                start=True,
                                stop=True,
                            )
                        else:
                            nc.tensor.matmul(
                                sc[:, h * 512:(h + 1) * 512],
                                lhsT=kpad[hp][h][i // 4][:, ko:ko + P],
                                rhs=q_q[hp][tq][:],
                                start=True,
                                stop=True,
                            )
                    schr = (bi, i) in SCHR
                    if schr:
                        exi = epool.tile([P, 1024], i32, tag="ei", name="ei")
                        nc.vector.tensor_scalar(
                            exi[:], sc[:], SCHR_A, SCHR_B,
                            mybir.AluOpType.mult, mybir.AluOpType.add,
                        )
                        # dtype-preserving f32r copy: the int bits ARE the
                        # exp value; walrus demands an f32r-typed producer
                        # for f32r matmul inputs
                        ex = epool.tile([P, 1024], f32r, tag="er", name="er")
                        nc.vector.tensor_copy(ex[:], exi[:].bitcast(f32r))
                    else:
                        ex = epool.tile([P, 1024], bf16, tag="e", name="e")
                        nc.scalar.activation(ex[:], sc[:], Exp, scale=0.125)
                    if pending is not None:
                        p = pending
                        emit_pv(p)
                        if p[4]:  # previous block's last pv -> normalize it
                            emit_norm(p[2], (bi - 1) % 4, p[0])
                    if i == 0:
                        pv = [
                            pvps.tile([P, 512], f32, tag=f"pv{h}", name=f"pv{h}")
                            for h in range(2)
                        ]
                    for fn in fillers.get((bi, i), ()):
                        fn()
                    pending = (pv, ex, hp, i, i == 15, schr)
                assert pending is not None
            emit_pv(pending)
            # junk matmuls keep HAM at full clock through the norm window
            # (measured: without them the tail outproj runs at 1.2 GHz)
            warm_mms(10)
            emit_norm(1, 3, pending[0])
            # tail outproj: the sc pool is free after the last exp, so each
            # unit gets a whole [128,1024] PSUM tile, one wide drain and one
            # wide row DMA; drains alternate between DVE and the idle ACT
            for tqc in range(4):
                row = 12 + tqc
                po = scps.tile([P, 1024], f32, tag="sc", name="sc")
                for d2 in range(2):
                    for hp in range(2):
                        nc.tensor.matmul(
                            po[:, d2 * 512:(d2 + 1) * 512],
                            lhsT=attn_q[hp][3][:, tqc * P:(tqc + 1) * P],
                            rhs=wo_sb[:, hp, d2 * 512:(d2 + 1) * 512],
                            start=(hp == 0),
                            stop=(hp == 1),
                        )
                ob = opool.tile([P, 1024], bf16, tag="ob2", name="ob2")
                if tqc % 2 == 0:
                    nc.vector.tensor_copy(ob[:], po[:])
                else:
                    nc.scalar.copy(ob[:], po[:])
                nc.sync.dma_start(out[row * P:(row + 1) * P, :], ob[:])

    nc.compile()
    return nc


def _get_module():
    if "nc" not in _CACHE:
        _CACHE["nc"] = _build_module()
    return _CACHE["nc"]


def make_in_maps(x, w_qkv, w_out):
    import ml_dtypes

    bf = ml_dtypes.bfloat16
    cos2, sin2 = _rope_tables_np()
    trig = np.ascontiguousarray(
        np.stack(
            [cos2.reshape(128, 4, 512), sin2.reshape(128, 4, 512)], axis=2
        )
    ).astype(bf)  # (128, 4, 2, 512)
    in_maps = []
    for c in range(NCORES):
        b, g = divmod(c, 4)
        q0 = 256 * g
        # column chunks: [q_hp0 | k_hp0 | q_hp1 | k_hp1]
        wqk_c = np.concatenate(
            [
                w_qkv[:, q0:q0 + 128],
                w_qkv[:, 1024 + q0:1024 + q0 + 128],
                w_qkv[:, q0 + 128:q0 + 256],
                w_qkv[:, 1024 + q0 + 128:1024 + q0 + 256],
            ],
            axis=1,
        )
        # (4cc, 2half, 128, 4dc, 128): per column-chunk, per dc-half
        wqk2 = np.ascontiguousarray(
            wqk_c.reshape(2, 4, 128, 4, 128).transpose(3, 0, 2, 1, 4)
        ).astype(bf)
        xt4 = np.ascontiguousarray(
            x[b].T.reshape(8, 128, 4, 512).transpose(2, 1, 0, 3)
        ).astype(bf)  # (4, 128, 8, 512)
        wv_c = np.ascontiguousarray(
            w_qkv[:, 2048 + q0:2048 + q0 + 256].reshape(8, 128, 256)
            .transpose(1, 0, 2)
        ).reshape(128, 2048)
        wo_c = np.ascontiguousarray(
            w_out[q0:q0 + 256, :].reshape(2, 128, 1024).transpose(1, 0, 2)
        ).reshape(128, 2048)
        wvo = np.ascontiguousarray(
            np.stack([wv_c, wo_c], axis=1)
        ).astype(bf)  # (128, 2, 2048)
        in_maps.append(
            {"xT": xt4, "w_qk": wqk2, "trig": trig, "wvo": wvo}
        )
    return in_maps


def combine_outputs(results, b_out):
    out = np.empty((B, T, D), dtype=np.float32)
    for b in range(B):
        acc = results[4 * b]["out"].astype(np.float32)
        for c in range(4 * b + 1, 4 * b + 4):
            acc += results[c]["out"].astype(np.float32)
        out[b] = acc + b_out[None, :]
    return out


def kernel(x, w_qkv, w_out, b_out, _trace=False, _tag=[0]):
    from concourse import bass_utils

    nc = _get_module()
    in_maps = make_in_maps(
        np.asarray(x, dtype=np.float32),
        np.asarray(w_qkv, dtype=np.float32),
        np.asarray(w_out, dtype=np.float32),
    )
    res = bass_utils.run_bass_kernel_spmd(
        nc, in_maps, core_ids=list(range(NCORES)), trace=_trace
    )
    if _trace:
        _CACHE["last_result"] = res
    return combine_outputs(res.results, np.asarray(b_out, dtype=np.float32))
